# revision 5
# baseline (speedup 1.0000x reference)
"""Trainium2 Bass kernel for nn_MultiHeadGATLayerMerged.

Math (reference semantics):
  Wh[d,h] = x @ W[d,h]                                   (per batch b)
  e_src[d] = x @ (W[d,H-1] @ a[d,H-1,:OUT])              (only last head's
  e_dst[d] = x @ (W[d,H-1] @ a[d,H-1,OUT:])               logits survive)
  z_d[i,j] = leakyrelu(e_src[d][i] + e_dst[d][j], 0.01)
  e[i,j]   = z_{d*}[i,j],  d* = max d with A_d[i,j] != 0, else -inf
  P = exp(e); alpha = P / rowsum(P)
  out = (1/H) * diag(1/rowsum) * sum_d (A_d o P) @ (x @ (W[d,0]+W[d,1]))

Kernel strategy (one NeuronCore per batch element, 8 cores):
  The (N,N) attention plane is computed TRANSPOSED (j on partitions, i on
  free axis).  Per j-chunk (128 partitions x 1024):
    - z_d = srcB_d + dst_d[j] on DVE tensor_scalar (4x mode)
    - priority merge: DVE copy_predicated onto a gpsimd-memset -3000 tile
    - Lrelu then Exp on ACT (batched in groups of 4 to amortize table loads)
    - S_d = mask_d * P: d=0,1 on DVE, d=2,3 on GpSimd (engine balance)
    - attention matmuls TRANSPOSED-OUTPUT: stationary whs (j,o) 128x128
      chunks, streaming S_d --> psum out_T[o, i].  8x fewer LDWEIGHTS.
    - rowsum via ones-column matmul streaming P
  Epilogue: transpose out_T back with PE identity matmuls, scale by
  1/(H*rowsum) via ACT per-partition scale, DMA out.
  Masks arrive as one packed (j, d, i) fp16 plane per j-chunk (one DMA).
"""

import numpy as np
import ml_dtypes
from contextlib import ExitStack

import concourse.bass as bass
import concourse.mybir as mybir
import concourse.tile as tile
from concourse import masks as cmasks
from concourse.bass_utils import run_bass_kernel_spmd

dt = mybir.dt
AF = mybir.ActivationFunctionType
ALU = mybir.AluOpType

B, N, F, OUT, H, D = 8, 1024, 256, 256, 2, 4
P = 128
NJC = N // P   # j chunks (partition chunks of transposed plane)
FC = F // P    # f chunks for contraction
GRP = 4        # jc group size for ACT table batching

# ---------------------------------------------------------------------------
# BIR post-pass: several walrus instruction structs in this toolchain have
# very few semaphore-wait slots (CopyPredicated: 0, Matmult: ~1, ...).
# Hoist waits into standalone EventSemaphore instructions placed immediately
# before the owning instruction (same engine, program order => semantics
# identical: waits are monotone thresholds).
_ZERO_WAIT_TYPES = (mybir.InstCopyPredicated,)
_DEFAULT_LIMIT = 1


def _hoist_waits(nc):
    n_fixed = 0
    for fn in nc.m.functions:
        for bb in fn.blocks:
            insts = list(bb.instructions)
            new_insts = []
            for inst in insts:
                si = getattr(inst, "sync_info", None)
                if si is not None and si.on_wait:
                    limit = 0 if isinstance(inst, _ZERO_WAIT_TYPES) else _DEFAULT_LIMIT
                    waits = list(si.on_wait)
                    if len(waits) > limit:
                        excess = waits[: len(waits) - limit] if limit else waits
                        keep = waits[len(waits) - limit:] if limit else []
                        for k in range(0, len(excess), 2):
                            ev = mybir.InstEventSemaphore(
                                name=f"{inst.name}-hw{k}", ins=[], outs=[])
                            ev.engine = inst.engine
                            ev.debug = inst.debug
                            ev.sync_info = mybir.SyncInfo(
                                on_wait=excess[k:k + 2], on_update=[])
                            new_insts.append(ev)
                        inst.sync_info = mybir.SyncInfo(
                            on_wait=keep, on_update=list(si.on_update))
                        n_fixed += 1
                new_insts.append(inst)
            bb.instructions = new_insts
    return n_fixed


def _dedupe_ldweights(nc):
    """Drop LDWEIGHTS identical to the immediately preceding LDWEIGHTS on PE
    (stationary operand persists across matmuls)."""
    n_drop = 0
    for fn in nc.m.functions:
        for bb in fn.blocks:
            insts = list(bb.instructions)
            new_insts = []
            prev_key = None
            for inst in insts:
                if isinstance(inst, mybir.InstLdweights):
                    key = repr(inst.ins[0])
                    si = getattr(inst, "sync_info", None)
                    clean = si is None or (not si.on_wait and not si.on_update)
                    if key == prev_key and clean:
                        n_drop += 1
                        continue
                    prev_key = key
                elif isinstance(inst, (mybir.InstMatmult, mybir.InstEventSemaphore)):
                    pass
                else:
                    if getattr(inst, "engine", None) == mybir.EngineType.PE:
                        prev_key = None
                new_insts.append(inst)
            bb.instructions = new_insts
    return n_drop


def _build(nc: bass.Bass):
    xT = nc.dram_tensor("xT", [F, N], dt.float32, kind="ExternalInput")
    # masks packed (j, d, i): one contiguous 8KB row per partition per chunk
    mJDI = nc.dram_tensor("mJDI", [N, D * N], dt.float16, kind="ExternalInput")
    wvec = nc.dram_tensor("wvec", [F, 2 * D], dt.float32, kind="ExternalInput")
    # summed-head weights packed (f, d*OUT)
    wsp = nc.dram_tensor("wsp", [F, D * OUT], dt.float32, kind="ExternalInput")
    out = nc.dram_tensor("out", [N, OUT], dt.float32, kind="ExternalOutput")

    with tile.TileContext(nc) as tc, ExitStack() as ctx:
        cpool = ctx.enter_context(tc.tile_pool(name="consts", bufs=1))
        spool = ctx.enter_context(tc.tile_pool(name="statics", bufs=1))

        onescol16 = cpool.tile([P, 1], dt.float16, name="onescol16", tag="onescol16")
        nc.vector.memset(onescol16[:], 1.0)
        onesrow = cpool.tile([1, P], dt.float32, name="onesrow", tag="onesrow")
        nc.vector.memset(onesrow[:], 1.0)
        ones11 = cpool.tile([1, 1], dt.float32, name="ones11", tag="ones11")
        nc.vector.memset(ones11[:], 1.0)
        ident = cpool.tile([P, P], dt.float32, name="ident", tag="ident")
        cmasks.make_identity(nc, ident[:])

        # ---- load x^T (f on partitions); round to fp32r for the PE
        xt = []
        xtr = []
        for fc in range(FC):
            t = spool.tile([P, N], dt.float32, name=f"xt{fc}", tag=f"xt{fc}")
            nc.sync.dma_start(t[:], xT[fc * P:(fc + 1) * P, :])
            xt.append(t)
            tr = spool.tile([P, N], dt.float32r, name=f"xtr{fc}", tag=f"xtr{fc}")
            nc.vector.tensor_copy(tr[:], t[:])
            xtr.append(tr[:])

        # ---- load wvec + packed Wsum
        wvr = []
        for fc in range(FC):
            t = spool.tile([P, 2 * D], dt.float32, name=f"wv{fc}", tag=f"wv{fc}")
            nc.sync.dma_start(t[:], wvec[fc * P:(fc + 1) * P, :])
            tr = spool.tile([P, 2 * D], dt.float32r, name=f"wvr{fc}", tag=f"wvr{fc}")
            nc.vector.tensor_copy(tr[:], t[:])
            wvr.append(tr[:])
        wsr = []
        for fc in range(FC):
            t = spool.tile([P, D * OUT], dt.float32, name=f"wsp{fc}", tag=f"wsp{fc}")
            nc.sync.dma_start(t[:], wsp[fc * P:(fc + 1) * P, :])
            tr = spool.tile([P, D * OUT], dt.float32r, name=f"wsr{fc}", tag=f"wsr{fc}")
            nc.gpsimd.tensor_copy(tr[:], t[:])
            wsr.append(tr[:])

        # ---- mask DMAs (packed per jc); pool rotates 4 buffers
        mpool = ctx.enter_context(tc.tile_pool(name="masks", bufs=4))
        mall = {}

        def issue_mask_dma(jc):
            t = mpool.tile([P, D * N], dt.float16, name=f"mall{jc}", tag="mall")
            nc.sync.dma_start(t[:], mJDI[jc * P:(jc + 1) * P, :])
            mall[jc] = t

        for jc in range(4):
            issue_mask_dma(jc)

        # ---- src rows (1, N) f32 per direction, then broadcast to fp16
        src_sb = []
        with tc.tile_pool(name="srcps", bufs=2, space="PSUM") as srcps:
            for d in range(D):
                ps = srcps.tile([1, N], dt.float32, name=f"sps{d}", tag="sps")
                for half in range(2):
                    sl = slice(half * 512, (half + 1) * 512)
                    for fc in range(FC):
                        nc.tensor.matmul(
                            ps[:, sl], wvr[fc][:, d:d + 1], xtr[fc][:, sl],
                            start=(fc == 0), stop=(fc == FC - 1))
                t = spool.tile([1, N], dt.float32, name=f"srcrow{d}", tag=f"srcrow{d}")
                nc.scalar.copy(t[:], ps[:])
                src_sb.append(t)
        onesrow_rt = cpool.tile([1, P], dt.float32r, name="onesrowr", tag="onesrowr")
        nc.vector.tensor_copy(onesrow_rt[:], onesrow[:])
        onesrow_r = onesrow_rt[:]
        srcb = []
        with tc.tile_pool(name="bcps", bufs=2, space="PSUM") as bcps:
            for d in range(D):
                srt = spool.tile([1, N], dt.float32r, name=f"srcr{d}", tag=f"srcr{d}")
                nc.vector.tensor_copy(srt[:], src_sb[d][:])
                sr = srt[:]
                ps = bcps.tile([P, N], dt.float32, name=f"bps{d}", tag="bps")
                for half in range(2):
                    sl = slice(half * 512, (half + 1) * 512)
                    nc.tensor.matmul(ps[:, sl], onesrow_r, sr[:, sl],
                                     start=True, stop=True)
                t = spool.tile([P, N], dt.float16, name=f"srcb{d}", tag=f"srcb{d}")
                nc.scalar.copy(t[:], ps[:])
                srcb.append(t)

        # ---- dst columns for ALL jc: (128, NJC*D) fp32
        dstc = spool.tile([P, NJC * D], dt.float32, name="dstc", tag="dstc")
        with tc.tile_pool(name="dstps", bufs=1, space="PSUM") as dstps:
            ps = dstps.tile([P, NJC * D], dt.float32, name="dps", tag="dps")
            for jc in range(NJC):
                for fc in range(FC):
                    nc.tensor.matmul(
                        ps[:, jc * D:(jc + 1) * D],
                        xtr[fc][:, jc * P:(jc + 1) * P], wvr[fc][:, D:2 * D],
                        start=(fc == 0), stop=(fc == FC - 1))
            nc.scalar.copy(dstc[:], ps[:])

        # ---- persistent PSUM: transposed output accumulators + rowsum
        outps_pool = ctx.enter_context(tc.tile_pool(name="outps", bufs=1, space="PSUM"))
        out_ps = [outps_pool.tile([P, N], dt.float32, name=f"ops{oh}", tag=f"ops{oh}")
                  for oh in range(2)]
        rs_pool = ctx.enter_context(ExitStack())
        rs_ps = rs_pool.enter_context(
            tc.tile_pool(name="rsps", bufs=1, space="PSUM")).tile(
                [1, N], dt.float32, name="rsps", tag="rsps")

        # ---- streaming pools
        loopctx = ctx.enter_context(ExitStack())
        projps = loopctx.enter_context(tc.tile_pool(name="projps", bufs=1, space="PSUM"))
        zpool = loopctx.enter_context(tc.tile_pool(name="zs", bufs=2))
        epool = loopctx.enter_context(tc.tile_pool(name="es", bufs=5))
        lpool = loopctx.enter_context(tc.tile_pool(name="lr", bufs=4))
        ppool = loopctx.enter_context(tc.tile_pool(name="ps16", bufs=4))
        sspool = loopctx.enter_context(tc.tile_pool(name="ss", bufs=2))
        wpool = loopctx.enter_context(tc.tile_pool(name="whsp", bufs=5))

        whs = {}

        def proj(jc):
            """Wh sums for all 4 directions of chunk jc -> whs[jc] fp16."""
            ps = projps.tile([P, D * OUT], dt.float32, name=f"pps{jc}", tag="pps")
            for fc in range(FC):
                for half in range(2):
                    sl = slice(half * 512, (half + 1) * 512)
                    nc.tensor.matmul(
                        ps[:, sl], xtr[fc][:, jc * P:(jc + 1) * P], wsr[fc][:, sl],
                        start=(fc == 0), stop=(fc == FC - 1),
                        skip_group_check=True)
            t = wpool.tile([P, D * OUT], dt.float16, name=f"whs{jc}", tag="whs")
            nc.scalar.copy(t[:], ps[:])
            whs[jc] = t

        for jc in range(GRP):
            proj(jc)

        # ================= main loop =================
        for g in range(NJC // GRP):
            jcs = tuple(range(GRP * g, GRP * (g + 1)))
            es = {}
            zs = {}
            pt = {}
            for jc in jcs:
                e = epool.tile([P, N], dt.float16, name="e", tag="e")
                nc.gpsimd.memset(e[:], -3000.0)
                zall = zpool.tile([P, D * N], dt.float16, name="zall", tag="zall")
                for d in range(D):
                    nc.vector.tensor_scalar(
                        zall[:, d * N:(d + 1) * N], srcb[d][:],
                        dstc[:, jc * D + d:jc * D + d + 1], None,
                        op0=ALU.add)
                mu = mall[jc][:].bitcast(dt.uint16)
                for d in range(D):
                    nc.vector.copy_predicated(
                        e[:], mu[:, d * N:(d + 1) * N], zall[:, d * N:(d + 1) * N])
                es[jc] = e
                zs[jc] = zall
            for jc in jcs:
                t = lpool.tile([P, N], dt.float16, name="elr", tag="elr")
                nc.scalar.activation(t[:], es[jc][:], AF.Lrelu, bias=0.0,
                                     scale=1.0, alpha=0.01)
                es[jc] = t
            for jc in jcs:
                p16 = ppool.tile([P, N], dt.float16, name="pt", tag="pt")
                nc.scalar.activation(p16[:], es[jc][:], AF.Exp, bias=0.0,
                                     scale=1.0)
                pt[jc] = p16

            for jc in jcs:
                # masks for jc+4 now that cp's of jc are done
                if jc + 4 < NJC:
                    issue_mask_dma(jc + 4)
                # S_d = mask_d * P: d=0,1 on DVE; d=2,3 on GpSimd
                sall = sspool.tile([P, D * N], dt.float16, name="sall", tag="sall")
                m = mall[jc]
                for d in range(2):
                    nc.vector.tensor_mul(sall[:, d * N:(d + 1) * N],
                                         m[:, d * N:(d + 1) * N], pt[jc][:])
                for d in range(2, 4):
                    nc.gpsimd.tensor_mul(sall[:, d * N:(d + 1) * N],
                                         m[:, d * N:(d + 1) * N], pt[jc][:])
                # projections for the next group (keeps PE fed)
                if jc + GRP < NJC:
                    proj(jc + GRP)
                # rowsum: rs[0, i] += sum_j P^T[j, i]
                for ih in range(2):
                    isl = slice(ih * 512, (ih + 1) * 512)
                    nc.tensor.matmul(rs_ps[:, isl], onescol16[:], pt[jc][:, isl],
                                     start=(jc == 0), stop=(jc == NJC - 1),
                                     skip_group_check=True)
                # transposed-output attention matmuls: stationary whs chunk
                w = whs[jc]
                for d in range(D):
                    for oh in range(2):
                        lhsT = w[:, d * OUT + oh * P: d * OUT + (oh + 1) * P]
                        for ih in range(2):
                            isl = slice(ih * 512, (ih + 1) * 512)
                            nc.tensor.matmul(
                                out_ps[oh][:, isl], lhsT,
                                sall[:, d * N + ih * 512: d * N + (ih + 1) * 512],
                                start=(jc == 0 and d == 0),
                                stop=(jc == NJC - 1 and d == D - 1),
                                skip_group_check=True)

        # ================= epilogue =================
        rs_sb = spool.tile([1, N], dt.float32, name="rssb", tag="rssb")
        nc.scalar.copy(rs_sb[:], rs_ps[:])
        loopctx.close()
        rs_pool.close()

        epips = ctx.enter_context(tc.tile_pool(name="epips", bufs=1, space="PSUM"))
        tpps = ctx.enter_context(tc.tile_pool(name="tpps", bufs=2, space="PSUM"))
        rsT_ps = epips.tile([P, NJC], dt.float32, name="rstp", tag="rstp")
        for ic in range(NJC):
            nc.tensor.matmul(rsT_ps[:, ic:ic + 1],
                             rs_sb[:, ic * P:(ic + 1) * P], ones11[:],
                             start=True, stop=True, skip_group_check=True)
        invH = spool.tile([P, NJC], dt.float32, name="invH", tag="invH")
        nc.vector.reciprocal(invH[:], rsT_ps[:])
        nc.vector.tensor_scalar_mul(invH[:], invH[:], 1.0 / H)

        outT_sb = []
        for oh in range(2):
            t = spool.tile([P, N], dt.float32, name=f"otsb{oh}", tag=f"otsb{oh}")
            nc.scalar.copy(t[:], out_ps[oh][:])
            outT_sb.append(t)

        opool = ctx.enter_context(tc.tile_pool(name="osb", bufs=3))
        for ic in range(NJC):
            tp = tpps.tile([P, OUT], dt.float32, name="tp", tag="tp")
            for oh in range(2):
                nc.tensor.transpose(tp[:, oh * P:(oh + 1) * P],
                                    outT_sb[oh][:, ic * P:(ic + 1) * P],
                                    ident[:])
            o = opool.tile([P, OUT], dt.float32, name="osb", tag="osb")
            nc.scalar.activation(o[:], tp[:], AF.Copy, bias=0.0,
                                 scale=invH[:, ic:ic + 1])
            nc.sync.dma_start(out[ic * P:(ic + 1) * P, :], o[:])

    return nc


_CACHED = {}


def _get_nc():
    if "nc" not in _CACHED:
        nc = bass.Bass()
        _build(nc)
        _hoist_waits(nc)
        _dedupe_ldweights(nc)
        _CACHED["nc"] = nc
    return _CACHED["nc"]


def _prep_shared(W, a):
    wv_cols = [W[d, H - 1] @ a[d, H - 1, :OUT] for d in range(D)] + \
              [W[d, H - 1] @ a[d, H - 1, OUT:] for d in range(D)]
    wvec = np.stack(wv_cols, axis=1).astype(np.float32)          # (F, 2D)
    ws = W.sum(axis=1)                                           # (D, F, OUT)
    wsp = np.ascontiguousarray(
        np.concatenate([ws[d] for d in range(D)], axis=1), dtype=np.float32)
    return wvec, wsp


def _prep_masks(A_U, A_D, A_R, A_L):
    masks_ = [np.asarray(m) for m in (A_U, A_D, A_R, A_L)]
    # transposed masks (j on rows): mT[d][j, i] = A_d[i, j]
    mT = np.stack([m.T for m in masks_])                         # (D, N, N)
    # packed (j, d, i) so each chunk is one contiguous DMA
    mjdi = np.ascontiguousarray(
        np.transpose(mT, (1, 0, 2)) != 0).astype(np.float16)     # (N, D, N)
    return mjdi.reshape(N, D * N)


def kernel(x, A_U, A_D, A_R, A_L, W, a):
    x = np.asarray(x, dtype=np.float32)
    W = np.asarray(W, dtype=np.float32)
    a = np.asarray(a, dtype=np.float32)

    m_jdi = _prep_masks(A_U, A_D, A_R, A_L)
    wvec, wsp = _prep_shared(W, a)

    nc = _get_nc()
    in_maps = []
    for b in range(B):
        in_maps.append({
            "xT": np.ascontiguousarray(x[b].T),
            "mJDI": m_jdi,
            "wvec": wvec,
            "wsp": wsp,
        })
    res = run_bass_kernel_spmd(nc, in_maps, list(range(B)))
    out = np.stack([res.results[b]["out"] for b in range(B)], axis=0)
    return out.astype(np.float32)
